# revision 13
# baseline (speedup 1.0000x reference)
"""Trainium2 Bass kernel for LogisticRegressionRBF.

reference:
    phi  = exp(-(||x_i||^2 + ||c_j||^2 - 2 x_i.c_j))   [K, N]
    out  = sigmoid(phi @ W.T + b)                      [K, 1]

K=16384, M=256 features, N=4096 centers, 8 NeuronCores, data-parallel over K
(2048 rows/core).

Structure (v3):
  - PE: one fp8e4m3 DoubleRow matmul per 512-col chunk computes the full
    256-deep contraction s*(x.c) in a single instruction (s = 256*log2e split
    as 19.217 into each operand), 4x the bf16 row rate.  psum is in "y128"
    units: y = 128*log2(e^{2 x.c}).
  - A1 columns (majority W sign, |W|,csq folded into the exponent by an f32r
    rank-1 aug matmul): one in-place ACT exp over psum with accum_out ->
    per-ktile partial sum.  1 engine-pass per element.
  - Weighted columns: exponent carries only 2x.c - xsq - G (per-partition
    bias); the per-column factor wc = W*e^{G-csq} (bf16) is applied
    post-exp: phi'' -> tt(*wc) -> one merged tensor_scalar accum (4x mode).
    phi'' produced either by ACT (exp -> bf16 stage) or by the DVE/u16
    Schraudolph trick: tensor_scalar (psum + bias)|max 0 -> uint16, whose
    bits read as bf16 equal 2^(y/128).  tt runs on DVE (2x bf16) or Pool.
  phi never touches HBM.
"""

import numpy as np

K_FULL = 16384
M_DIM = 256
N_DIM = 4096
N_CORES = 8
K_SHARD = K_FULL // N_CORES          # 2048
K_TILES = K_SHARD // 128             # 16
N_FREE = 512

LOG2E = 1.4426950408889634
SCH_C = 126.94269504                  # Schraudolph floor-bias constant

# Column layout in psum, two half-psum "generations" per ktile so consumers
# of one half never block the matmuls of the other (all multiples of 512):
#   gen0 (banks 0-3): [A1a | D5D]    gen1 (banks 4-7): [A1b | D5P]
# A1* = ACT exp+accum in place with PE rank-1 aug (majority-sign cols)
# D5D = DVE u16-exp + DVE tt * wc;  D5P = DVE u16-exp + Pool tt * wc
W_A1A = 1536
W_D5D = 512
W_A1B = 1024
W_D5P = 1024
W_A1 = W_A1A + W_A1B
W_WCT = W_D5D + W_D5P          # weighted (wc) columns
assert W_A1A + W_D5D == 2048 and W_A1B + W_D5P == 2048
assert all(w % 512 == 0 for w in (W_A1A, W_D5D, W_A1B, W_D5P))

_PATCHED = False


def _patch_tile_drain():
    """This container's walrus allows max 1 semaphore wait per instruction
    (2 for EventSemaphore); TileContext's kernel-tail drain collects every
    outstanding semaphore on one Drain and codegen dies with "Too many sync
    wait commands".  Redistribute: one single-wait NOP per semaphore, then a
    waitless drain."""
    global _PATCHED
    if _PATCHED:
        return
    import concourse.mybir as mybir
    import concourse.tile as tile

    def _drain_and_barrier(self, tick_clock, wait_clock):
        from concourse.tile import ScopedClock

        nc = self.nc
        probe = nc.sync.nop(nofuse=True, hint="tile_drain_waits")
        wait_clock.add_sem_waits(
            probe.ins, ScopedClock({None: tick_clock.global_clock})
        )
        waits = list(probe.ins.sync_info.on_wait)
        del probe.ins.sync_info.on_wait[:]
        if waits:
            probe.ins.sync_info.on_wait.append(waits[0])
            for w in waits[1:]:
                n = nc.sync.nop(nofuse=True, hint="tile_drain_waits")
                if n.ins.sync_info is None:
                    n.ins.sync_info = mybir.SyncInfo(on_wait=[], on_update=[])
                n.ins.sync_info.on_wait.append(w)
        nc.sync.drain()

        nc.all_engine_barrier()
        assert self.sems is not None
        popped = nc._tile_sem_poison_stack.pop()
        assert popped is self._sem_poison
        nc.clear_and_free_semaphores(list(self.sems.allocated().values()))
        nc.all_engine_barrier()

    tile.TileContext._drain_and_barrier = _drain_and_barrier
    _PATCHED = True


def _split_excess_waits(nc):
    """Walrus in this container accepts at most 1 semaphore wait per
    instruction (2 for EventSemaphore), but Tile's scheduler emits up to 3.
    Hoist the excess into single-wait NOPs just before the instruction on the
    same engine -- per-engine program order makes this equivalent."""
    import concourse.mybir as mybir

    fn = nc.m.functions[0]
    n_split = 0
    for bb in fn.blocks:
        new_insts = []
        for inst in bb.instructions:
            si = inst.sync_info
            cap = 2 if inst.opcode == "EventSemaphore" else 1
            if si is not None and len(si.on_wait) > cap:
                extras = list(si.on_wait[cap:])
                del si.on_wait[cap:]
                for i, w in enumerate(extras):
                    nop = mybir.InstNoOp(
                        name=f"{inst.name}_sw{i}",
                        engine=inst.engine,
                        sync_info=mybir.SyncInfo(on_wait=[w], on_update=[]),
                        text_hint="split_wait",
                        bass_nofuse=True,
                    )
                    nc.register_instruction(nop)
                    new_insts.append(nop)
                    n_split += 1
            new_insts.append(inst)
        bb.instructions[:] = new_insts
    return n_split


def build_program(meta):
    """Emit the per-core Bass program (SPMD: identical on all 8 cores)."""
    import concourse.bass as bass
    import concourse.mybir as mybir
    import concourse.tile as tile
    from concourse.alu_op_type import AluOpType

    _patch_tile_drain()
    f32 = mybir.dt.float32
    f32r = mybir.dt.float32r
    bf16 = mybir.dt.bfloat16
    u16 = mybir.dt.uint16
    fp8 = mybir.dt.float8e4
    AF = mybir.ActivationFunctionType
    DR = mybir.MatmulPerfMode.DoubleRow

    b_val = meta["b_val"]
    sign_a = meta["sign_a"]
    sign_b = meta["sign_b"]

    nc = bass.Bass()
    x8_d = nc.dram_tensor("x8", [128, 2 * K_SHARD], fp8, kind="ExternalInput")
    c8_d = nc.dram_tensor("c8", [128, 2 * N_DIM], fp8, kind="ExternalInput")
    wc_d = nc.dram_tensor("wc", [128, W_WCT], bf16, kind="ExternalInput")
    aug_d = nc.dram_tensor("aug", [1, W_A1], f32r, kind="ExternalInput")
    ones_d = nc.dram_tensor("ones", [1, 128], f32r, kind="ExternalInput")
    biasa1_d = nc.dram_tensor("biasa1", [128, K_TILES], f32, kind="ExternalInput")
    biasd_d = nc.dram_tensor("biasd", [128, K_TILES], f32, kind="ExternalInput")
    out_d = nc.dram_tensor("out", [K_SHARD, 1], f32, kind="ExternalOutput")

    with tile.TileContext(nc) as tc:
        with (
            tc.tile_pool(name="const", bufs=1) as cpool,
            tc.tile_pool(name="psum", bufs=1, space=bass.MemorySpace.PSUM) as ppool,
            tc.tile_pool(name="small", bufs=2) as spool,
        ):
            x8_s = cpool.tile([128, 2 * K_SHARD], fp8, tag="x8")
            c8_s = cpool.tile([128, 2 * N_DIM], fp8, tag="c8")
            wc_s = cpool.tile([128, W_WCT], bf16, tag="wc")
            aug_s = cpool.tile([1, W_A1], f32r, tag="aug")
            ones_s = cpool.tile([1, 128], f32r, tag="ones")
            biasa1_s = cpool.tile([128, K_TILES], f32, tag="biasa1")
            biasd_s = cpool.tile([128, K_TILES], f32, tag="biasd")
            partials = cpool.tile([128, 4 * K_TILES], f32, tag="partials")

            # DMA order = consumption order (single demand-ordered queue)
            nc.sync.dma_start(x8_s[:, 0:128], x8_d[:, 0:128])
            nc.sync.dma_start(
                x8_s[:, K_SHARD : K_SHARD + 128],
                x8_d[:, K_SHARD : K_SHARD + 128],
            )
            nc.sync.dma_start(ones_s[:], ones_d[:])
            nc.sync.dma_start(biasa1_s[:], biasa1_d[:])
            nc.sync.dma_start(biasd_s[:], biasd_d[:])
            nc.sync.dma_start(c8_s[:, 0:2048], c8_d[:, 0:2048])
            nc.sync.dma_start(
                c8_s[:, N_DIM : N_DIM + 2048], c8_d[:, N_DIM : N_DIM + 2048]
            )
            nc.sync.dma_start(aug_s[:], aug_d[:])
            nc.sync.dma_start(wc_s[:, 0:W_D5D], wc_d[:, 0:W_D5D])
            nc.sync.dma_start(c8_s[:, 2048:4096], c8_d[:, 2048:4096])
            nc.sync.dma_start(
                c8_s[:, N_DIM + 2048 : N_DIM + 4096],
                c8_d[:, N_DIM + 2048 : N_DIM + 4096],
            )
            nc.sync.dma_start(wc_s[:, W_D5D:W_WCT], wc_d[:, W_D5D:W_WCT])
            nc.sync.dma_start(x8_s[:, 128:K_SHARD], x8_d[:, 128:K_SHARD])
            nc.sync.dma_start(
                x8_s[:, K_SHARD + 128 :], x8_d[:, K_SHARD + 128 :]
            )

            x8_v = x8_s[:].rearrange("p (i k) -> p i k", i=2)
            c8_v = c8_s[:].rearrange("p (i n) -> p i n", i=2)

            ps = ppool.tile([128, N_DIM], f32, tag="ps", name="ps_all")

            # psum chunk -> (is_a1, aug offset) for the rank-1 fold
            # layout: [A1a | D5D | A1b | D5P] = [0:1536 | :2048 | :3072 | :4096]
            chunk_a1 = {0: 0, 1: 512, 2: 1024, 4: W_A1A, 5: W_A1A + 512}

            # deferred Pool-side sums (DVE never waits on Pool)
            pool_jk = []

            def flush_pool_sum():
                if not pool_jk:
                    return
                jk_p, tp = pool_jk.pop(0)
                nc.vector.tensor_scalar(
                    jk_p[:, W_D5D:W_WCT],
                    jk_p[:, W_D5D:W_WCT],
                    1.0,
                    None,
                    AluOpType.mult,
                    AluOpType.add,
                    accum_out=partials[:, 3 * K_TILES + tp : 3 * K_TILES + tp + 1],
                )

            def mm(t, q):
                kc = slice(t * 128, (t + 1) * 128)
                lo = q * N_FREE
                a1_off = chunk_a1.get(q)
                nc.tensor.matmul(
                    ps[:, lo : lo + N_FREE],
                    x8_v[:, :, kc],
                    c8_v[:, :, lo : lo + N_FREE],
                    start=True,
                    stop=a1_off is None,
                    perf_mode=DR,
                    skip_group_check=True,
                )
                if a1_off is not None:
                    nc.tensor.matmul(
                        ps[:, lo : lo + N_FREE],
                        ones_s[:, 0:128],
                        aug_s[:, a1_off : a1_off + N_FREE],
                        start=False,
                        stop=True,
                        skip_group_check=True,
                    )

            scale = float(np.log(2.0) / 128.0)
            for t in range(K_TILES):
                jk = spool.tile([128, W_WCT], bf16, tag="jk", bufs=3,
                                name=f"jk_{t}")
                stgd = spool.tile([128, W_WCT], u16, tag="stgd", bufs=3,
                                  name=f"stgd_{t}")
                flush_pool_sum()

                # --- gen0: banks 0-3 = [A1a | D5D] ---
                for q in range(4):
                    mm(t, q)
                nc.scalar.activation(
                    ps[:, 0:W_A1A],
                    ps[:, 0:W_A1A],
                    AF.Exp,
                    bias=biasa1_s[:, t : t + 1],
                    scale=scale,
                    accum_out=partials[:, t : t + 1],
                )
                nc.vector.tensor_scalar(
                    stgd[:, 0:W_D5D],
                    ps[:, W_A1A : W_A1A + W_D5D],
                    biasd_s[:, t : t + 1],
                    0.0,
                    AluOpType.add,
                    AluOpType.max,
                )
                nc.vector.tensor_tensor(
                    jk[:, 0:W_D5D],
                    stgd[:, 0:W_D5D].bitcast(bf16),
                    wc_s[:, 0:W_D5D],
                    AluOpType.mult,
                )

                # --- gen1: banks 4-7 = [A1b | D5P] ---
                for q in range(4, 8):
                    mm(t, q)
                nc.scalar.activation(
                    ps[:, 2048 : 2048 + W_A1B],
                    ps[:, 2048 : 2048 + W_A1B],
                    AF.Exp,
                    bias=biasa1_s[:, t : t + 1],
                    scale=scale,
                    accum_out=partials[:, K_TILES + t : K_TILES + t + 1],
                )
                nc.vector.tensor_scalar(
                    stgd[:, W_D5D:W_WCT],
                    ps[:, 2048 + W_A1B : N_DIM],
                    biasd_s[:, t : t + 1],
                    0.0,
                    AluOpType.add,
                    AluOpType.max,
                )
                nc.gpsimd.tensor_tensor(
                    jk[:, W_D5D:W_WCT],
                    stgd[:, W_D5D:W_WCT].bitcast(bf16),
                    wc_s[:, W_D5D:W_WCT],
                    AluOpType.mult,
                )
                # DVE-side sum of the D5D product
                nc.vector.tensor_scalar(
                    jk[:, 0:W_D5D],
                    jk[:, 0:W_D5D],
                    1.0,
                    None,
                    AluOpType.mult,
                    AluOpType.add,
                    accum_out=partials[:, 2 * K_TILES + t : 2 * K_TILES + t + 1],
                )
                pool_jk.append((jk, t))
            flush_pool_sum()

            # --- combine partials -> logits -> sigmoid ---
            logits = spool.tile([128, K_TILES], f32, tag="logits")
            tmp = spool.tile([128, K_TILES], f32, tag="tmp")
            sig = spool.tile([128, K_TILES], f32, tag="sig")

            for lo_t, hi_t in ((0, K_TILES - 1), (K_TILES - 1, K_TILES)):
                cs = slice(lo_t, hi_t)

                def plane(s):
                    return partials[:, s * K_TILES + lo_t : s * K_TILES + hi_t]

                nc.vector.tensor_add(logits[:, cs], plane(2), plane(3))
                nc.vector.scalar_tensor_tensor(
                    tmp[:, cs], plane(0), float(sign_a), logits[:, cs],
                    AluOpType.mult, AluOpType.add,
                )
                nc.vector.scalar_tensor_tensor(
                    logits[:, cs], plane(1), float(sign_b), tmp[:, cs],
                    AluOpType.mult, AluOpType.add,
                )
                # sigmoid(z + b) = 1 / (1 + exp(-(z + b)))
                nc.scalar.activation(
                    tmp[:, cs], logits[:, cs], AF.Exp, bias=float(-b_val),
                    scale=-1.0,
                )
                nc.vector.tensor_scalar_add(tmp[:, cs], tmp[:, cs], 1.0)
                nc.vector.reciprocal(sig[:, cs], tmp[:, cs])
            out_v = out_d.rearrange("(p t) o -> p (t o)", p=128)
            nc.sync.dma_start(out_v[:, :], sig[:, :])

    _split_excess_waits(nc)
    return nc


def _host_prep(x, x_basis, W, b):
    import ml_dtypes

    w = np.asarray(W, np.float64).reshape(-1)
    xb = np.asarray(x_basis, np.float64)
    csq = (xb * xb).sum(axis=1)

    # column permutation: A1a/A1b are single-sign groups (|W| folded into the
    # exponent; sign applied at combine); the rest are weighted by signed wc.
    # psum layout [A1a | D5D | A1b | D5P]
    n_pos = int((w >= 0).sum())
    pos = np.flatnonzero(w >= 0)
    neg = np.flatnonzero(w < 0)
    if n_pos >= W_A1A and N_DIM - n_pos >= W_A1B:
        a_cols, b_cols, sign_a, sign_b = pos, neg, 1.0, -1.0
    elif N_DIM - n_pos >= W_A1A and n_pos >= W_A1B:
        a_cols, b_cols, sign_a, sign_b = neg, pos, -1.0, 1.0
    else:
        raise ValueError(f"degenerate sign split n_pos={n_pos}")
    a1a = a_cols[:W_A1A]
    a1b = b_cols[:W_A1B]
    wcols = np.concatenate([a_cols[W_A1A:], b_cols[W_A1B:]])
    perm = np.concatenate([
        a1a, wcols[0:W_D5D], a1b, wcols[W_D5D:W_WCT],
    ])
    xb_p = xb[perm]

    # A1 aug: exponent gets (ln|W| - csq) via rank-1, in y128 units, ordered
    # [A1a cols | A1b cols]
    a1_cols = np.concatenate([a1a, a1b])
    with np.errstate(divide="ignore"):
        lnw = np.maximum(np.log(np.abs(w[a1_cols])), -1e30)
    aug = ((lnw - csq[a1_cols]) * (128.0 * LOG2E)).astype(np.float32)
    aug = aug.reshape(1, W_A1)

    # weighted cols: global shift G so wc = W e^(G-csq) sits in range,
    # ordered [D5D cols | D5P cols]
    cw = csq[wcols]
    G = float((cw.min() + cw.max()) / 2.0 - 7.5)
    with np.errstate(over="ignore", under="ignore"):
        wc_row = (w[wcols] * np.exp(G - cw)).astype(np.float32)
    wc_row = np.nan_to_num(wc_row, posinf=3e38, neginf=-3e38)
    wc = np.ascontiguousarray(
        np.broadcast_to(wc_row, (128, W_WCT))
    ).astype(ml_dtypes.bfloat16)

    # fp8 matmul operands, scale sqrt(2*log2e*128) on each side
    s_half = float(np.sqrt(2.0 * LOG2E * 128.0))
    c8 = np.zeros((128, 2 * N_DIM), ml_dtypes.float8_e4m3)
    xb_s = (xb_p * s_half).astype(ml_dtypes.float8_e4m3)
    for h in range(2):
        c8[:, h * N_DIM : (h + 1) * N_DIM] = xb_s[:, h * 128 : (h + 1) * 128].T

    x64 = np.asarray(x, np.float64)
    xsq = (x64 * x64).sum(axis=1)
    b_val = float(np.asarray(b).reshape(-1)[0])

    dperm = (np.arange(K_SHARD) % 128) * K_TILES + (np.arange(K_SHARD) // 128)

    per_core = []
    for c in range(N_CORES):
        sl = slice(c * K_SHARD, (c + 1) * K_SHARD)
        xs8 = (x64[sl][dperm] * s_half).astype(ml_dtypes.float8_e4m3)
        x8 = np.zeros((128, 2 * K_SHARD), ml_dtypes.float8_e4m3)
        for h in range(2):
            x8[:, h * K_SHARD : (h + 1) * K_SHARD] = (
                xs8[:, h * 128 : (h + 1) * 128].T
            )
        xsq_pt = xsq[sl].reshape(128, K_TILES)
        biasa1 = (-xsq_pt).astype(np.float32)
        biasa4 = (-(xsq_pt + G)).astype(np.float32)
        biasd = ((-(xsq_pt + G) * LOG2E + SCH_C) * 128.0).astype(np.float32)
        per_core.append({
            "x8": x8.view(np.uint8), "c8": c8.view(np.uint8), "wc": wc,
            "aug": aug, "ones": np.ones((1, 128), np.float32),
            "biasa1": biasa1, "biasa4": biasa4, "biasd": biasd,
        })
    meta = {"b_val": b_val, "sign_a": sign_a, "sign_b": sign_b}
    return per_core, meta


def kernel(x, x_basis, W, b):
    from concourse.bass_utils import run_bass_kernel_spmd

    in_maps, meta = _host_prep(x, x_basis, W, b)
    nc = build_program(meta)
    res = run_bass_kernel_spmd(nc, in_maps, core_ids=list(range(N_CORES)))
    out = np.concatenate([r["out"] for r in res.results], axis=0)
    return out.astype(np.float32)
